# revision 36
# baseline (speedup 1.0000x reference)
"""Multi-head self-attention (B=4, L=2048, D=1024, H=16, RoPE, causal) on 8
Trainium2 NeuronCores.

Sharding: data-parallel over batch (4) x tensor-parallel over head groups (2).
Core i handles batch i//2, heads 8*(i%2) .. 8*(i%2)+8.  Each core computes its
QKV projection slice, RoPE, causal attention for its 8 heads, and a partial
output projection over its 512 d-columns; the host sums the two partials per
batch.

v2 changes vs the 325us baseline (now ~316us):
  - exp split across engines: diagonal k-tiles (need causal masks) use the
    scalar ACT exact exp + DVE post-mask; half the off-diagonal k-tiles
    compute exp on the DVE via the Schraudolph bit trick: the int16 result
    of (S*23.083 + 16252.66) written into the bf16 pt buffer IS
    bf16(exp2(S*0.125*log2e)) to +-3%.  Softmax normalization cancels the
    common-mode part of that error; measured output error is unchanged.
    This halves the scalar-engine exp load (166us -> ~110us), which was
    rate-limiting the attention pipeline through st-PSUM buffer recycling.
  - jq=0 attention rounds (S^T + exp, all-diagonal) are emitted during the
    QKV phase (st PSUM pool opened outermost) so the scalar engine's idle
    QKV window absorbs 16 of the 64 diagonal exps.
  - y-projection PSUM->SBUF copies alternate between DVE and scalar so
    neither queue delays the exp stream.
  - prologue DMAs issue in parallel across the sync/scalar/gpsimd queues
    (first matmul at ~10us instead of ~15.5us).
  Rejected after measurement: fp8 DoubleRow Q/K projection (rel err
  1.9e-2: logit-magnitude-scaled quantization error defeats softmax
  cancellation on sharp heads; the USE_FP8_QK=False path remains), fp8
  early-pt tiles (rel err 1.5e-2), gpsimd masks/yt (PSUM unreadable from
  gpsimd; mask latency lands on the diag critical path), plain DVE
  reciprocal (3.4us/call), DVE divide (walrus rejects).
"""
import sys
sys.path.insert(0, "/opt/trn_rl_repo")

import numpy as np
import ml_dtypes

B, L, D, H = 4, 2048, 1024, 16
DH = D // H  # 64
THETA = 100000.0
NCORES = 8
BF = ml_dtypes.bfloat16
F8 = ml_dtypes.float8_e4m3

# Schraudolph exp2 trick constants (bf16 bit pattern via int16):
#   bf16_bits(exp(S * 0.125)) ~= int16(S * A16 + B16)
A16 = 0.125 * 1.4426950408889634 * 128.0   # 23.0831...
B16 = (127.0 - 0.0303) * 128.0 + 0.5       # minimax sigma + truncation comp

USE_FP8_QK = False
USE_SCHRAUDOLPH = True

_built = None


def _rope_tables():
    # [128, L]: rows = 2 stacked heads' dh (64 each), identical per head.
    # Scaled by 2^-9 to dequantize the fp8 qk projection (x*2^3, W*2^6).
    pos = np.arange(L, dtype=np.float32)
    inv_freq = (1.0 / THETA ** (np.arange(0, DH, 2, dtype=np.float32) / DH))
    ang = pos[None, :] * inv_freq[:, None]              # [32, L]
    cos = np.cos(ang)
    sin = np.sin(ang)
    cos2 = np.repeat(cos, 2, axis=0)                     # rows 2p,2p+1 = cos_p
    sin2 = np.empty((DH, L), np.float32)
    sin2[0::2] = -sin
    sin2[1::2] = sin
    s = 2.0 ** -9 if USE_FP8_QK else 1.0
    return (np.concatenate([cos2, cos2], 0) * s,
            np.concatenate([sin2, sin2], 0) * s)


def _build():
    import concourse.mybir as mybir
    import concourse.tile as tile
    from concourse import bacc

    FP32 = mybir.dt.float32
    BF16 = mybir.dt.bfloat16
    FP8 = mybir.dt.float8e4
    INT16 = mybir.dt.int16
    MUL = mybir.AluOpType.mult
    ADD = mybir.AluOpType.add
    DIV = mybir.AluOpType.divide
    EXP = mybir.ActivationFunctionType.Exp
    DR = mybir.MatmulPerfMode.DoubleRow
    SWAP_MASK = [i ^ 1 for i in range(32)]

    nc = bacc.Bacc(None, target_bir_lowering=False)
    # DRAM parameters (per-core shapes; host prepares layouts)
    xt_d = nc.declare_dram_parameter("xt", [8, 128, L], BF16, False)          # x^T d-chunks (V proj)
    if USE_FP8_QK:
        xt8_d = nc.declare_dram_parameter("xt8", [4, 128, 2, L], FP8, False)
        wqk8_d = nc.declare_dram_parameter("wqk8", [4, 8, 128, 2, 128], FP8, False)
    else:
        wqk_d = nc.declare_dram_parameter("wqk", [8, 8, 128, 128], BF16, False)
    wv_d = nc.declare_dram_parameter("wv", [8, 128, 512], BF16, False)        # [dchunk, d, e_v]
    wo_d = nc.declare_dram_parameter("wo", [4, 2, 128, 512], BF16, False)     # [dchunk, ehalf, d, e]
    cos_d = nc.declare_dram_parameter("cos2", [128, L], BF16, False)
    sin_d = nc.declare_dram_parameter("sin2", [128, L], BF16, False)
    tri_d = nc.declare_dram_parameter("trimask", [128, 128], BF16, False)
    y_d = nc.declare_dram_parameter("y", [L, D], BF16, True)

    with tile.TileContext(nc) as tc:
        import contextlib
        ctx = contextlib.ExitStack()
        with ctx:
            # ---- resident SBUF pools (bufs=1: one slot per tag) ----
            res = ctx.enter_context(tc.tile_pool(name="res", bufs=1))
            wq_pool = ctx.enter_context(tc.tile_pool(name="wqk", bufs=16))
            rope_pool = ctx.enter_context(tc.tile_pool(name="rope", bufs=3))
            pt_pool = ctx.enter_context(tc.tile_pool(name="pt", bufs=8))
            rec_pool = ctx.enter_context(tc.tile_pool(name="rec", bufs=2))
            y_pool = ctx.enter_context(tc.tile_pool(name="yt", bufs=4))

            xt = [res.tile([128, L], BF16, tag=f"xt{d}", name=f"xt{d}") for d in range(8)]
            if USE_FP8_QK:
                xt8 = [res.tile([128, 2, L], FP8, tag=f"xt8{d}", name=f"xt8{d}")
                       for d in range(4)]
            qkr = [res.tile([128, L], BF16, tag=f"qkr{c}", name=f"qkr{c}") for c in range(8)]
            vsb = [res.tile([128, 512], BF16, tag=f"v{t}", name=f"v{t}") for t in range(16)]
            wv_sb = [res.tile([128, 512], BF16, tag=f"wv{d}", name=f"wv{d}") for d in range(8)]
            wo_sb = [res.tile([128, 512], BF16, tag=f"wo{i}", name=f"wo{i}") for i in range(8)]
            cos_sb = res.tile([128, L], BF16, tag="cos")
            sin_sb = res.tile([128, L], BF16, tag="sin")
            tri_sb = res.tile([128, 128], BF16, tag="tri")
            ones_sb = res.tile([128, DH], BF16, tag="ones")

            nc.vector.memset(ones_sb, 1.0)

            def load_w8(c):
                wts = []
                if USE_FP8_QK:
                    for dc in range(4):
                        w = wq_pool.tile([128, 2, 128], FP8, tag="w",
                                         name=f"w8_{c}_{dc}")
                        nc.sync.dma_start(out=w, in_=wqk8_d[dc, c])
                        wts.append(w)
                else:
                    for d in range(8):
                        w = wq_pool.tile([128, 128], BF16, tag="w",
                                         name=f"w_{c}_{d}")
                        nc.sync.dma_start(out=w, in_=wqk_d[d, c])
                        wts.append(w)
                return wts

            def emit_qk_l4(ps_pool, c, wts, l4):
                    lsl = slice(512 * l4, 512 * l4 + 512)
                    qkp = ps_pool.tile([128, 512], FP32, tag="qkps")
                    if USE_FP8_QK:
                        for dc in range(4):
                            nc.tensor.matmul(qkp, wts[dc], xt8[dc][:, :, lsl],
                                             start=(dc == 0), stop=(dc == 3),
                                             perf_mode=DR)
                    else:
                        for d in range(8):
                            nc.tensor.matmul(qkp, wts[d], xt[d][:, lsl],
                                             start=(d == 0), stop=(d == 7))
                    # rope: qkr[c][:,lsl] = qkp*cos + swap(qkp)*sin
                    shf = rope_pool.tile([128, 512], FP32, tag="shf")
                    nc.vector.stream_shuffle(shf, qkp, SWAP_MASK)
                    t1 = rope_pool.tile([128, 512], FP32, tag="t1")
                    nc.vector.tensor_tensor(out=t1, in0=qkp, in1=cos_sb[:, lsl], op=MUL)
                    t2 = rope_pool.tile([128, 512], FP32, tag="t2")
                    nc.vector.tensor_tensor(out=t2, in0=shf, in1=sin_sb[:, lsl], op=MUL)
                    nc.gpsimd.tensor_tensor(out=qkr[c][:, lsl], in0=t1, in1=t2, op=ADD)

            def emit_qk_chunk(ps_pool, c, wts=None):
                """QKV projection for qk e-chunk c (128 e-cols) + RoPE."""
                if wts is None:
                    wts = load_w8(c)
                for l4 in range(4):
                    emit_qk_l4(ps_pool, c, wts, l4)

            def emit_v_tile(ps_pool, t):
                vp = ps_pool.tile([128, 512], FP32, tag="vps")
                lsl = slice(128 * t, 128 * t + 128)
                for d in range(8):
                    nc.tensor.matmul(vp, xt[d][:, lsl], wv_sb[d],
                                     start=(d == 0), stop=(d == 7))
                nc.scalar.copy(out=vsb[t], in_=vp)

            dve_ctr = [0]

            def emit_st_exp(st_ps, jq, p, k, pt, force_scalar=False):
                """S^T matmuls + exp for k-tile k of round (jq,p) into pt."""
                qb0 = 512 * jq
                kpos = 128 * k
                vs = max(0, kpos - qb0)
                qt, kt = qkr[p], qkr[4 + p]
                st = st_ps.tile([128, 1024], FP32, tag="st")
                ksl = slice(kpos, kpos + 128)
                qsl = slice(qb0 + vs, qb0 + 512)
                nc.tensor.matmul(st[:, vs:512], kt[0:64, ksl],
                                 qt[0:64, qsl], start=True, stop=True,
                                 tile_position=(0, 0))
                nc.tensor.matmul(st[:, 512 + vs:1024], kt[64:128, ksl],
                                 qt[64:128, qsl], start=True, stop=True,
                                 tile_position=(64, 0))
                use_dve = (kpos < qb0 and USE_SCHRAUDOLPH
                           and not force_scalar and (dve_ctr[0] % 2 == 0))
                if kpos < qb0 and USE_SCHRAUDOLPH and not force_scalar:
                    dve_ctr[0] += 1
                if not use_dve:
                    nc.scalar.activation(out=pt[:, vs:1024], in_=st[:, vs:1024],
                                         func=EXP, scale=0.125)
                    if kpos >= qb0:
                        dsl = slice(vs, vs + 128)
                        dslb = slice(512 + vs, 512 + vs + 128)
                        nc.vector.tensor_tensor(out=pt[:, dsl], in0=pt[:, dsl],
                                                in1=tri_sb, op=MUL)
                        nc.vector.tensor_tensor(out=pt[:, dslb], in0=pt[:, dslb],
                                                in1=tri_sb, op=MUL)
                else:
                    # off-diagonal: Schraudolph bf16-bit exp on the DVE
                    nc.vector.tensor_scalar(
                        out=pt[:, 0:1024].bitcast(INT16), in0=st[:, 0:1024],
                        scalar1=A16, scalar2=B16, op0=MUL, op1=ADD)
                return (k, pt, vs)

            # ---- phase layout ----
            # st PSUM pool opens OUTERMOST (banks 0-3) so jq=0 attention
            # rounds' S^T+exp run during the QKV phase; qk/v pools (banks
            # 4-7) close before av/s/y open in the same banks (LIFO).
            ot = [res.tile([128, L], BF16, tag=f"ot{p}", name=f"ot{p}") for p in range(4)]
            pend = {}

            with tc.tile_pool(name="ps_st", bufs=2, space="PSUM") as st_ps:
                with tc.tile_pool(name="ps_qk", bufs=2, space="PSUM") as qk_ps, \
                     tc.tile_pool(name="ps_v", bufs=2, space="PSUM") as v_ps:
                    # prologue DMAs: qk-chunk-0 weights + xt8 first so the
                    # first matmul starts ~1.6us in.
                    # prologue: spread first-need DMAs across engine queues
                    # so issue doesn't serialize on the sync engine.
                    engs = [nc.sync, nc.scalar, nc.gpsimd]
                    w0 = load_w8(0)
                    if USE_FP8_QK:
                        for d in range(4):
                            engs[d % 2 + 1].dma_start(out=xt8[d], in_=xt8_d[d])
                    else:
                        for d in range(8):
                            engs[d % 2 + 1].dma_start(out=xt[d][:, 0:512],
                                                      in_=xt_d[d][:, 0:512])
                    w4 = load_w8(4)
                    for d in range(8):
                        engs[d % 2 + 1].dma_start(out=xt[d][:, 512:1024],
                                                  in_=xt_d[d][:, 512:1024])
                    nc.scalar.dma_start(out=cos_sb, in_=cos_d[:, :])
                    nc.gpsimd.dma_start(out=sin_sb, in_=sin_d[:, :])
                    for d in range(8):
                        engs[d % 2 + 1].dma_start(out=xt[d][:, 1024:2048],
                                                  in_=xt_d[d][:, 1024:2048])
                    for d in range(8):
                        engs[d % 3].dma_start(out=wv_sb[d], in_=wv_d[d])
                    nc.gpsimd.dma_start(out=tri_sb, in_=tri_d[:, :])

                    def early_round(jq, p):
                        # S^T + exact exp emitted during the QKV phase;
                        # consumed by the sav chains in phase 2.
                        lst = []
                        for k in range(4 * (jq + 1)):
                            pt = res.tile([128, 1024], BF16,
                                          tag=f"pte{jq}_{p}_{k}")
                            lst.append(emit_st_exp(st_ps, jq, p, k, pt,
                                                   force_scalar=True))
                        pend[(jq, p)] = lst

                    emit_qk_l4(qk_ps, 0, w0, 0)
                    emit_qk_l4(qk_ps, 4, w4, 0)
                    emit_qk_l4(qk_ps, 0, w0, 1)
                    emit_qk_l4(qk_ps, 4, w4, 1)
                    early_round(0, 0)
                    emit_qk_l4(qk_ps, 0, w0, 2)
                    emit_qk_l4(qk_ps, 4, w4, 2)
                    emit_qk_l4(qk_ps, 0, w0, 3)
                    emit_qk_l4(qk_ps, 4, w4, 3)
                    for t in range(4):
                        emit_v_tile(v_ps, t)
                    emit_qk_chunk(qk_ps, 1)
                    emit_qk_chunk(qk_ps, 5)
                    early_round(0, 1)
                    emit_qk_chunk(qk_ps, 2)
                    emit_qk_chunk(qk_ps, 6)
                    early_round(0, 2)
                    for t in range(4, 16):
                        emit_v_tile(v_ps, t)
                    for dc in range(4):
                        for eh in range(2):
                            nc.sync.dma_start(out=wo_sb[dc * 2 + eh],
                                              in_=wo_d[dc, eh])
                    emit_qk_chunk(qk_ps, 3)
                    emit_qk_chunk(qk_ps, 7)
                    early_round(0, 3)

                with tc.tile_pool(name="ps_av", bufs=1, space="PSUM") as av_ps, \
                     tc.tile_pool(name="ps_s", bufs=1, space="PSUM") as s_ps, \
                     tc.tile_pool(name="ps_y", bufs=2, space="PSUM") as y_ps:
                    pending_norm = [None]
                    proj_queue = []

                    def emit_norm(p, qb0, av, s, n):
                        scr = rec_pool.tile([128, 512], FP32, tag="lns",
                                            name=f"lns{n}")
                        rs = rec_pool.tile([128, 512], FP32, tag="rs",
                                           name=f"rs{n}")
                        nc.vector.reciprocal_approx_fast(out=rs, in_=s)
                        nc.vector.tensor_tensor(out=ot[p][:, qb0:qb0 + 512],
                                                in0=av, in1=rs, op=MUL)

                    def emit_proj_half(t, eh):
                        lsl = slice(128 * t, 128 * t + 128)
                        yp = y_ps.tile([128, 512], FP32, tag="yps")
                        for dc in range(4):
                            nc.tensor.matmul(yp, ot[dc][:, lsl],
                                             wo_sb[dc * 2 + eh],
                                             start=(dc == 0), stop=(dc == 3))
                        yt = y_pool.tile([128, 512], BF16, tag="yt")
                        if eh == 0:
                            nc.vector.tensor_copy(out=yt, in_=yp)
                        else:
                            nc.scalar.copy(out=yt, in_=yp)
                        nc.sync.dma_start(
                            out=y_d[lsl, 512 * eh:512 * eh + 512], in_=yt)

                    def make_sav(p, jq, nk, av, s):
                        def emit_sav(k, pt, vs):
                            first, last = (k == 0), (k == nk - 1)
                            isl = slice(vs, 512)
                            bsl = slice(512 + vs, 1024)
                            vca = 128 * p
                            vcb = 128 * p + 64
                            nc.tensor.matmul(s[0:64, isl], ones_sb, pt[:, isl],
                                             start=first, stop=last,
                                             tile_position=(0, 0),
                                             skip_group_check=True)
                            nc.tensor.matmul(s[64:128, isl], ones_sb, pt[:, bsl],
                                             start=first, stop=last,
                                             tile_position=(0, 64),
                                             skip_group_check=True)
                            nc.tensor.matmul(av[0:64, isl],
                                             vsb[k][:, vca:vca + 64], pt[:, isl],
                                             start=first, stop=last,
                                             tile_position=(0, 0),
                                             skip_group_check=True)
                            nc.tensor.matmul(av[64:128, isl],
                                             vsb[k][:, vcb:vcb + 64], pt[:, bsl],
                                             start=first, stop=last,
                                             tile_position=(0, 64),
                                             skip_group_check=True)
                        return emit_sav

                    rounds = [(jq, p) for jq in range(4) for p in range(4)]
                    for n, (jq, p) in enumerate(rounds):
                        qb0 = 512 * jq
                        nk = 4 * (jq + 1)
                        av = av_ps.tile([128, 512], FP32, tag="av", name=f"av{n}")
                        s = s_ps.tile([128, 512], FP32, tag="s", name=f"s{n}")
                        sav = make_sav(p, jq, nk, av, s)
                        SKEW = 5
                        pending = []
                        if jq == 0:
                            if pending_norm[0] is not None:
                                pending_norm[0]()
                                pending_norm[0] = None
                            for args in pend[(jq, p)]:
                                sav(*args)
                        else:
                            for k in range(nk):
                                pt = pt_pool.tile([128, 1024], BF16, tag="pt")
                                args = emit_st_exp(st_ps, jq, p, k, pt)
                                if k == 1 and pending_norm[0] is not None:
                                    pending_norm[0]()
                                    pending_norm[0] = None
                                if proj_queue and k >= 2:
                                    emit_proj_half(*proj_queue.pop(0))
                                pending.append(args)
                                if len(pending) > SKEW:
                                    sav(*pending.pop(0))
                            for args in pending[:len(pending)]:
                                sav(*args)
                        pending_norm[0] = (lambda p=p, qb0=qb0, av=av, s=s, n=n:
                                           emit_norm(p, qb0, av, s, n))
                        if p == 3:
                            pending_norm[0]()
                            pending_norm[0] = None
                            for t in range(4 * jq, 4 * jq + 4):
                                for eh in range(2):
                                    proj_queue.append((t, eh))
                    for t, eh in proj_queue:
                        emit_proj_half(t, eh)
    nc.compile()
    return nc


def _get_nc():
    global _built
    if _built is None:
        _built = _build()
    return _built


def _in_maps(x, W, Wo):
    x = np.asarray(x, np.float32)
    W = np.asarray(W, np.float32)
    Wo = np.asarray(Wo, np.float32)

    cos2, sin2 = _rope_tables()
    cos2 = cos2.astype(BF)
    sin2 = sin2.astype(BF)
    tri = np.zeros((128, 128), np.float32)
    p_idx = np.arange(128)
    tri[p_idx[:, None] <= p_idx[None, :]] = 1.0  # valid: k <= q
    tri = tri.astype(BF)

    in_maps = []
    for core in range(NCORES):
        b, g = core // 2, core % 2
        xtf = np.ascontiguousarray(x[b].T)                           # [D, L]
        xt = xtf.astype(BF).reshape(8, 128, L)
        wq = W[512 * g:512 * g + 512]                                # [512, D]
        wk = W[D + 512 * g:D + 512 * g + 512]
        wv = W[2 * D + 512 * g:2 * D + 512 * g + 512]
        wqk_t = np.ascontiguousarray(
            np.concatenate([wq, wk], 0).T)                           # [D, 1024]
        wv_t = np.ascontiguousarray(wv.T).astype(BF).reshape(8, 128, 512)
        wo_t = np.ascontiguousarray(Wo[:, 512 * g:512 * g + 512].T).astype(BF)  # [512, D]
        wo_t = wo_t.reshape(4, 128, 2, 512).transpose(0, 2, 1, 3)
        wo_t = np.ascontiguousarray(wo_t)
        m = {"xt": xt, "wv": wv_t, "wo": wo_t,
             "cos2": cos2, "sin2": sin2, "trimask": tri}
        if USE_FP8_QK:
            x8 = np.clip(xtf * 8.0, -240, 240).astype(F8)            # 2^3
            m["xt8"] = np.ascontiguousarray(
                x8.reshape(4, 2, 128, L).transpose(0, 2, 1, 3))      # [dc,128,2,L]
            w8 = np.clip(wqk_t * 64.0, -240, 240).astype(F8)         # 2^6
            m["wqk8"] = np.ascontiguousarray(
                w8.reshape(4, 2, 128, 8, 128).transpose(0, 3, 2, 1, 4))
        else:
            m["wqk"] = np.ascontiguousarray(
                wqk_t.astype(BF).reshape(8, 128, 8, 128).transpose(0, 2, 1, 3))
        in_maps.append(m)
    return in_maps


def kernel(x, W, Wo):
    from concourse.bass_utils import run_bass_kernel_spmd

    res = run_bass_kernel_spmd(_get_nc(), _in_maps(x, W, Wo),
                               list(range(NCORES)))
    out = np.empty((B, L, D), np.float32)
    for b in range(B):
        out[b] = (res.results[2 * b]["y"].astype(np.float32)
                  + res.results[2 * b + 1]["y"].astype(np.float32))
    return out


def _install_ntff_hook_shim():
    """The trimmed repo lacks antenv.axon_hooks; reconstruct it so
    run_bass_kernel_spmd(trace=True) can NTFF-profile through axon."""
    import sys as _sys, types
    if "antenv.axon_hooks" in _sys.modules:
        return
    import antenv  # noqa: F401
    from trn_agent_boot.trn_boot import _ntff_profile_via_ctypes
    hook = _ntff_profile_via_ctypes("/opt/axon/libaxon_pjrt.so")
    mod = types.ModuleType("antenv.axon_hooks")
    mod.set_axon_ntff_profile_hook = lambda h: None
    mod.get_axon_ntff_profile_hook = lambda: hook
    _sys.modules["antenv.axon_hooks"] = mod


def kernel_traced(x, W, Wo, tmpdir=None):
    """Run with NTFF tracing; returns BassKernelResults (trace in tmpdir)."""
    from concourse.bass_utils import run_bass_kernel_spmd

    _install_ntff_hook_shim()
    res = run_bass_kernel_spmd(_get_nc(), _in_maps(x, W, Wo),
                               list(range(NCORES)), trace=True, tmpdir=tmpdir)
    return res.exec_time_ns


# revision 37
# speedup vs baseline: 1.0080x; 1.0080x over previous
"""Multi-head self-attention (B=4, L=2048, D=1024, H=16, RoPE, causal) on 8
Trainium2 NeuronCores.

Sharding: data-parallel over batch (4) x tensor-parallel over head groups (2).
Core i handles batch i//2, heads 8*(i%2) .. 8*(i%2)+8.  Each core computes its
QKV projection slice, RoPE, causal attention for its 8 heads, and a partial
output projection over its 512 d-columns; the host sums the two partials per
batch.

v2 changes vs the 325us baseline (now ~316us):
  - exp split across engines: diagonal k-tiles (need causal masks) use the
    scalar ACT exact exp + DVE post-mask; half the off-diagonal k-tiles
    compute exp on the DVE via the Schraudolph bit trick: the int16 result
    of (S*23.083 + 16252.66) written into the bf16 pt buffer IS
    bf16(exp2(S*0.125*log2e)) to +-3%.  Softmax normalization cancels the
    common-mode part of that error; measured output error is unchanged.
    This halves the scalar-engine exp load (166us -> ~110us), which was
    rate-limiting the attention pipeline through st-PSUM buffer recycling.
  - jq=0 attention rounds (S^T + exp, all-diagonal) are emitted during the
    QKV phase (st PSUM pool opened outermost) so the scalar engine's idle
    QKV window absorbs 16 of the 64 diagonal exps.
  - y-projection PSUM->SBUF copies alternate between DVE and scalar so
    neither queue delays the exp stream.
  - prologue DMAs issue in parallel across the sync/scalar/gpsimd queues
    (first matmul at ~10us instead of ~15.5us).
  Rejected after measurement: fp8 DoubleRow Q/K projection (rel err
  1.9e-2: logit-magnitude-scaled quantization error defeats softmax
  cancellation on sharp heads; the USE_FP8_QK=False path remains), fp8
  early-pt tiles (rel err 1.5e-2), gpsimd masks/yt (PSUM unreadable from
  gpsimd; mask latency lands on the diag critical path), plain DVE
  reciprocal (3.4us/call), DVE divide (walrus rejects).
"""
import sys
sys.path.insert(0, "/opt/trn_rl_repo")

import numpy as np
import ml_dtypes

B, L, D, H = 4, 2048, 1024, 16
DH = D // H  # 64
THETA = 100000.0
NCORES = 8
BF = ml_dtypes.bfloat16
F8 = ml_dtypes.float8_e4m3

# Schraudolph exp2 trick constants (bf16 bit pattern via int16):
#   bf16_bits(exp(S * 0.125)) ~= int16(S * A16 + B16)
A16 = 0.125 * 1.4426950408889634 * 128.0   # 23.0831...
B16 = (127.0 - 0.0303) * 128.0 + 0.5       # minimax sigma + truncation comp

USE_FP8_QK = False
USE_SCHRAUDOLPH = True

_built = None


def _rope_tables():
    # [128, L]: rows = 2 stacked heads' dh (64 each), identical per head.
    # Scaled by 2^-9 to dequantize the fp8 qk projection (x*2^3, W*2^6).
    pos = np.arange(L, dtype=np.float32)
    inv_freq = (1.0 / THETA ** (np.arange(0, DH, 2, dtype=np.float32) / DH))
    ang = pos[None, :] * inv_freq[:, None]              # [32, L]
    cos = np.cos(ang)
    sin = np.sin(ang)
    cos2 = np.repeat(cos, 2, axis=0)                     # rows 2p,2p+1 = cos_p
    sin2 = np.empty((DH, L), np.float32)
    sin2[0::2] = -sin
    sin2[1::2] = sin
    s = 2.0 ** -9 if USE_FP8_QK else 1.0
    return (np.concatenate([cos2, cos2], 0) * s,
            np.concatenate([sin2, sin2], 0) * s)


def _build():
    import concourse.mybir as mybir
    import concourse.tile as tile
    from concourse import bacc

    FP32 = mybir.dt.float32
    BF16 = mybir.dt.bfloat16
    FP8 = mybir.dt.float8e4
    INT16 = mybir.dt.int16
    MUL = mybir.AluOpType.mult
    ADD = mybir.AluOpType.add
    DIV = mybir.AluOpType.divide
    EXP = mybir.ActivationFunctionType.Exp
    DR = mybir.MatmulPerfMode.DoubleRow
    SWAP_MASK = [i ^ 1 for i in range(32)]

    nc = bacc.Bacc(None, target_bir_lowering=False)
    # DRAM parameters (per-core shapes; host prepares layouts)
    xt_d = nc.declare_dram_parameter("xt", [8, 128, L], BF16, False)          # x^T d-chunks (V proj)
    if USE_FP8_QK:
        xt8_d = nc.declare_dram_parameter("xt8", [4, 128, 2, L], FP8, False)
        wqk8_d = nc.declare_dram_parameter("wqk8", [4, 8, 128, 2, 128], FP8, False)
    else:
        wqk_d = nc.declare_dram_parameter("wqk", [8, 8, 128, 128], BF16, False)
    wv_d = nc.declare_dram_parameter("wv", [8, 128, 512], BF16, False)        # [dchunk, d, e_v]
    wo_d = nc.declare_dram_parameter("wo", [4, 2, 128, 512], BF16, False)     # [dchunk, ehalf, d, e]
    cos_d = nc.declare_dram_parameter("cos2", [128, L], BF16, False)
    sin_d = nc.declare_dram_parameter("sin2", [128, L], BF16, False)
    tri_d = nc.declare_dram_parameter("trimask", [128, 128], BF16, False)
    y_d = nc.declare_dram_parameter("y", [L, D], BF16, True)

    with tile.TileContext(nc) as tc:
        import contextlib
        ctx = contextlib.ExitStack()
        with ctx:
            # ---- resident SBUF pools (bufs=1: one slot per tag) ----
            res = ctx.enter_context(tc.tile_pool(name="res", bufs=1))
            wq_pool = ctx.enter_context(tc.tile_pool(name="wqk", bufs=16))
            rope_pool = ctx.enter_context(tc.tile_pool(name="rope", bufs=2))
            pt_pool = ctx.enter_context(tc.tile_pool(name="pt", bufs=8))
            rec_pool = ctx.enter_context(tc.tile_pool(name="rec", bufs=2))
            y_pool = ctx.enter_context(tc.tile_pool(name="yt", bufs=4))

            xt = [res.tile([128, L], BF16, tag=f"xt{d}", name=f"xt{d}") for d in range(8)]
            if USE_FP8_QK:
                xt8 = [res.tile([128, 2, L], FP8, tag=f"xt8{d}", name=f"xt8{d}")
                       for d in range(4)]
            qkr = [res.tile([128, L], BF16, tag=f"qkr{c}", name=f"qkr{c}") for c in range(8)]
            vsb = [res.tile([128, 512], BF16, tag=f"v{t}", name=f"v{t}") for t in range(16)]
            wv_sb = [res.tile([128, 512], BF16, tag=f"wv{d}", name=f"wv{d}") for d in range(8)]
            wo_sb = [res.tile([128, 512], BF16, tag=f"wo{i}", name=f"wo{i}") for i in range(8)]
            cos_sb = res.tile([128, L], BF16, tag="cos")
            sin_sb = res.tile([128, L], BF16, tag="sin")
            tri_sb = res.tile([128, 128], BF16, tag="tri")
            ones_sb = res.tile([128, DH], BF16, tag="ones")

            nc.vector.memset(ones_sb, 1.0)

            def load_w8(c):
                wts = []
                if USE_FP8_QK:
                    for dc in range(4):
                        w = wq_pool.tile([128, 2, 128], FP8, tag="w",
                                         name=f"w8_{c}_{dc}")
                        nc.sync.dma_start(out=w, in_=wqk8_d[dc, c])
                        wts.append(w)
                else:
                    for d in range(8):
                        w = wq_pool.tile([128, 128], BF16, tag="w",
                                         name=f"w_{c}_{d}")
                        nc.sync.dma_start(out=w, in_=wqk_d[d, c])
                        wts.append(w)
                return wts

            def emit_qk_l4(ps_pool, c, wts, l4):
                    lsl = slice(512 * l4, 512 * l4 + 512)
                    qkp = ps_pool.tile([128, 512], FP32, tag="qkps")
                    if USE_FP8_QK:
                        for dc in range(4):
                            nc.tensor.matmul(qkp, wts[dc], xt8[dc][:, :, lsl],
                                             start=(dc == 0), stop=(dc == 3),
                                             perf_mode=DR)
                    else:
                        for d in range(8):
                            nc.tensor.matmul(qkp, wts[d], xt[d][:, lsl],
                                             start=(d == 0), stop=(d == 7))
                    # rope: qkr[c][:,lsl] = qkp*cos + swap(qkp)*sin
                    shf = rope_pool.tile([128, 512], FP32, tag="shf")
                    nc.vector.stream_shuffle(shf, qkp, SWAP_MASK)
                    t1 = rope_pool.tile([128, 512], FP32, tag="t1")
                    nc.vector.tensor_tensor(out=t1, in0=qkp, in1=cos_sb[:, lsl], op=MUL)
                    t2 = rope_pool.tile([128, 512], FP32, tag="t2")
                    nc.vector.tensor_tensor(out=t2, in0=shf, in1=sin_sb[:, lsl], op=MUL)
                    nc.gpsimd.tensor_tensor(out=qkr[c][:, lsl], in0=t1, in1=t2, op=ADD)

            def emit_qk_chunk(ps_pool, c, wts=None):
                """QKV projection for qk e-chunk c (128 e-cols) + RoPE."""
                if wts is None:
                    wts = load_w8(c)
                for l4 in range(4):
                    emit_qk_l4(ps_pool, c, wts, l4)

            def emit_v_tile(ps_pool, t):
                vp = ps_pool.tile([128, 512], FP32, tag="vps")
                lsl = slice(128 * t, 128 * t + 128)
                for d in range(8):
                    nc.tensor.matmul(vp, xt[d][:, lsl], wv_sb[d],
                                     start=(d == 0), stop=(d == 7))
                nc.scalar.copy(out=vsb[t], in_=vp)

            dve_ctr = [0]

            def emit_st_exp(st_ps, jq, p, k, pt, force_scalar=False):
                """S^T matmuls + exp for k-tile k of round (jq,p) into pt."""
                qb0 = 512 * jq
                kpos = 128 * k
                vs = max(0, kpos - qb0)
                qt, kt = qkr[p], qkr[4 + p]
                st = st_ps.tile([128, 1024], FP32, tag="st")
                ksl = slice(kpos, kpos + 128)
                qsl = slice(qb0 + vs, qb0 + 512)
                nc.tensor.matmul(st[:, vs:512], kt[0:64, ksl],
                                 qt[0:64, qsl], start=True, stop=True,
                                 tile_position=(0, 0))
                nc.tensor.matmul(st[:, 512 + vs:1024], kt[64:128, ksl],
                                 qt[64:128, qsl], start=True, stop=True,
                                 tile_position=(64, 0))
                use_dve = (kpos < qb0 and USE_SCHRAUDOLPH
                           and not force_scalar and (dve_ctr[0] % 2 == 0))
                if kpos < qb0 and USE_SCHRAUDOLPH and not force_scalar:
                    dve_ctr[0] += 1
                if not use_dve:
                    nc.scalar.activation(out=pt[:, vs:1024], in_=st[:, vs:1024],
                                         func=EXP, scale=0.125)
                    if kpos >= qb0:
                        dsl = slice(vs, vs + 128)
                        dslb = slice(512 + vs, 512 + vs + 128)
                        nc.vector.tensor_tensor(out=pt[:, dsl], in0=pt[:, dsl],
                                                in1=tri_sb, op=MUL)
                        nc.vector.tensor_tensor(out=pt[:, dslb], in0=pt[:, dslb],
                                                in1=tri_sb, op=MUL)
                else:
                    # off-diagonal: Schraudolph bf16-bit exp on the DVE
                    nc.vector.tensor_scalar(
                        out=pt[:, 0:1024].bitcast(INT16), in0=st[:, 0:1024],
                        scalar1=A16, scalar2=B16, op0=MUL, op1=ADD)
                return (k, pt, vs)

            # ---- phase layout ----
            # st PSUM pool opens OUTERMOST (banks 0-3) so jq=0 attention
            # rounds' S^T+exp run during the QKV phase; qk/v pools (banks
            # 4-7) close before av/s/y open in the same banks (LIFO).
            ot = [res.tile([128, L], BF16, tag=f"ot{p}", name=f"ot{p}") for p in range(4)]
            pend = {}

            with tc.tile_pool(name="ps_st", bufs=2, space="PSUM") as st_ps:
                with tc.tile_pool(name="ps_qk", bufs=2, space="PSUM") as qk_ps, \
                     tc.tile_pool(name="ps_v", bufs=2, space="PSUM") as v_ps:
                    # prologue DMAs: qk-chunk-0 weights + xt8 first so the
                    # first matmul starts ~1.6us in.
                    # prologue: spread first-need DMAs across engine queues
                    # so issue doesn't serialize on the sync engine.
                    engs = [nc.sync, nc.scalar, nc.gpsimd]
                    w0 = load_w8(0)
                    if USE_FP8_QK:
                        for d in range(4):
                            engs[d % 2 + 1].dma_start(out=xt8[d], in_=xt8_d[d])
                    else:
                        for d in range(8):
                            engs[d % 2 + 1].dma_start(out=xt[d][:, 0:512],
                                                      in_=xt_d[d][:, 0:512])
                    w4 = load_w8(4)
                    for d in range(8):
                        engs[d % 2 + 1].dma_start(out=xt[d][:, 512:1024],
                                                  in_=xt_d[d][:, 512:1024])
                    nc.scalar.dma_start(out=cos_sb, in_=cos_d[:, :])
                    nc.gpsimd.dma_start(out=sin_sb, in_=sin_d[:, :])
                    for d in range(8):
                        engs[d % 2 + 1].dma_start(out=xt[d][:, 1024:2048],
                                                  in_=xt_d[d][:, 1024:2048])
                    for d in range(8):
                        engs[d % 3].dma_start(out=wv_sb[d], in_=wv_d[d])
                    nc.gpsimd.dma_start(out=tri_sb, in_=tri_d[:, :])

                    def early_round(jq, p):
                        # S^T + exact exp emitted during the QKV phase;
                        # consumed by the sav chains in phase 2.
                        lst = []
                        for k in range(4 * (jq + 1)):
                            pt = res.tile([128, 1024], BF16,
                                          tag=f"pte{jq}_{p}_{k}")
                            lst.append(emit_st_exp(st_ps, jq, p, k, pt,
                                                   force_scalar=True))
                        pend[(jq, p)] = lst

                    emit_qk_l4(qk_ps, 0, w0, 0)
                    emit_qk_l4(qk_ps, 4, w4, 0)
                    emit_qk_l4(qk_ps, 0, w0, 1)
                    emit_qk_l4(qk_ps, 4, w4, 1)
                    early_round(0, 0)
                    emit_qk_l4(qk_ps, 0, w0, 2)
                    emit_qk_l4(qk_ps, 4, w4, 2)
                    emit_qk_l4(qk_ps, 0, w0, 3)
                    emit_qk_l4(qk_ps, 4, w4, 3)
                    for t in range(4):
                        emit_v_tile(v_ps, t)
                    emit_qk_chunk(qk_ps, 1)
                    emit_qk_chunk(qk_ps, 5)
                    early_round(0, 1)
                    emit_qk_chunk(qk_ps, 2)
                    emit_qk_chunk(qk_ps, 6)
                    early_round(0, 2)
                    for t in range(4, 16):
                        emit_v_tile(v_ps, t)
                    for dc in range(4):
                        for eh in range(2):
                            nc.sync.dma_start(out=wo_sb[dc * 2 + eh],
                                              in_=wo_d[dc, eh])
                    emit_qk_chunk(qk_ps, 3)
                    emit_qk_chunk(qk_ps, 7)
                    early_round(0, 3)

                with tc.tile_pool(name="ps_av", bufs=1, space="PSUM") as av_ps, \
                     tc.tile_pool(name="ps_s", bufs=1, space="PSUM") as s_ps, \
                     tc.tile_pool(name="ps_y", bufs=2, space="PSUM") as y_ps:
                    pending_norm = [None]
                    proj_queue = []

                    def emit_norm(p, qb0, av, s, n):
                        scr = rec_pool.tile([128, 512], FP32, tag="lns",
                                            name=f"lns{n}")
                        rs = rec_pool.tile([128, 512], FP32, tag="rs",
                                           name=f"rs{n}")
                        nc.vector.reciprocal_approx_fast(out=rs, in_=s)
                        nc.vector.tensor_tensor(out=ot[p][:, qb0:qb0 + 512],
                                                in0=av, in1=rs, op=MUL)

                    def emit_proj_half(t, eh):
                        lsl = slice(128 * t, 128 * t + 128)
                        yp = y_ps.tile([128, 512], FP32, tag="yps")
                        for dc in range(4):
                            nc.tensor.matmul(yp, ot[dc][:, lsl],
                                             wo_sb[dc * 2 + eh],
                                             start=(dc == 0), stop=(dc == 3))
                        yt = y_pool.tile([128, 512], BF16, tag="yt")
                        if eh == 0:
                            nc.vector.tensor_copy(out=yt, in_=yp)
                        else:
                            nc.scalar.copy(out=yt, in_=yp)
                        nc.sync.dma_start(
                            out=y_d[lsl, 512 * eh:512 * eh + 512], in_=yt)

                    def make_sav(p, jq, nk, av, s):
                        def emit_sav(k, pt, vs):
                            first, last = (k == 0), (k == nk - 1)
                            isl = slice(vs, 512)
                            bsl = slice(512 + vs, 1024)
                            vca = 128 * p
                            vcb = 128 * p + 64
                            nc.tensor.matmul(s[0:64, isl], ones_sb, pt[:, isl],
                                             start=first, stop=last,
                                             tile_position=(0, 0),
                                             skip_group_check=True)
                            nc.tensor.matmul(s[64:128, isl], ones_sb, pt[:, bsl],
                                             start=first, stop=last,
                                             tile_position=(0, 64),
                                             skip_group_check=True)
                            nc.tensor.matmul(av[0:64, isl],
                                             vsb[k][:, vca:vca + 64], pt[:, isl],
                                             start=first, stop=last,
                                             tile_position=(0, 0),
                                             skip_group_check=True)
                            nc.tensor.matmul(av[64:128, isl],
                                             vsb[k][:, vcb:vcb + 64], pt[:, bsl],
                                             start=first, stop=last,
                                             tile_position=(0, 64),
                                             skip_group_check=True)
                        return emit_sav

                    rounds = [(jq, p) for jq in range(4) for p in range(4)]
                    for n, (jq, p) in enumerate(rounds):
                        qb0 = 512 * jq
                        nk = 4 * (jq + 1)
                        av = av_ps.tile([128, 512], FP32, tag="av", name=f"av{n}")
                        s = s_ps.tile([128, 512], FP32, tag="s", name=f"s{n}")
                        sav = make_sav(p, jq, nk, av, s)
                        SKEW = 5
                        pending = []
                        if jq == 0:
                            if pending_norm[0] is not None:
                                pending_norm[0]()
                                pending_norm[0] = None
                            for args in pend[(jq, p)]:
                                sav(*args)
                        else:
                            for k in range(nk):
                                pt = pt_pool.tile([128, 1024], BF16, tag="pt")
                                args = emit_st_exp(st_ps, jq, p, k, pt)
                                if k == 1 and pending_norm[0] is not None:
                                    pending_norm[0]()
                                    pending_norm[0] = None
                                if proj_queue and k >= 2:
                                    emit_proj_half(*proj_queue.pop(0))
                                pending.append(args)
                                if len(pending) > SKEW:
                                    sav(*pending.pop(0))
                            for args in pending[:len(pending)]:
                                sav(*args)
                        pending_norm[0] = (lambda p=p, qb0=qb0, av=av, s=s, n=n:
                                           emit_norm(p, qb0, av, s, n))
                        if p == 3:
                            pending_norm[0]()
                            pending_norm[0] = None
                            for t in range(4 * jq, 4 * jq + 4):
                                for eh in range(2):
                                    proj_queue.append((t, eh))
                    for t, eh in proj_queue:
                        emit_proj_half(t, eh)
    nc.compile()
    return nc


def _get_nc():
    global _built
    if _built is None:
        _built = _build()
    return _built


def _in_maps(x, W, Wo):
    x = np.asarray(x, np.float32)
    W = np.asarray(W, np.float32)
    Wo = np.asarray(Wo, np.float32)

    cos2, sin2 = _rope_tables()
    cos2 = cos2.astype(BF)
    sin2 = sin2.astype(BF)
    tri = np.zeros((128, 128), np.float32)
    p_idx = np.arange(128)
    tri[p_idx[:, None] <= p_idx[None, :]] = 1.0  # valid: k <= q
    tri = tri.astype(BF)

    in_maps = []
    for core in range(NCORES):
        b, g = core // 2, core % 2
        xtf = np.ascontiguousarray(x[b].T)                           # [D, L]
        xt = xtf.astype(BF).reshape(8, 128, L)
        wq = W[512 * g:512 * g + 512]                                # [512, D]
        wk = W[D + 512 * g:D + 512 * g + 512]
        wv = W[2 * D + 512 * g:2 * D + 512 * g + 512]
        wqk_t = np.ascontiguousarray(
            np.concatenate([wq, wk], 0).T)                           # [D, 1024]
        wv_t = np.ascontiguousarray(wv.T).astype(BF).reshape(8, 128, 512)
        wo_t = np.ascontiguousarray(Wo[:, 512 * g:512 * g + 512].T).astype(BF)  # [512, D]
        wo_t = wo_t.reshape(4, 128, 2, 512).transpose(0, 2, 1, 3)
        wo_t = np.ascontiguousarray(wo_t)
        m = {"xt": xt, "wv": wv_t, "wo": wo_t,
             "cos2": cos2, "sin2": sin2, "trimask": tri}
        if USE_FP8_QK:
            x8 = np.clip(xtf * 8.0, -240, 240).astype(F8)            # 2^3
            m["xt8"] = np.ascontiguousarray(
                x8.reshape(4, 2, 128, L).transpose(0, 2, 1, 3))      # [dc,128,2,L]
            w8 = np.clip(wqk_t * 64.0, -240, 240).astype(F8)         # 2^6
            m["wqk8"] = np.ascontiguousarray(
                w8.reshape(4, 2, 128, 8, 128).transpose(0, 3, 2, 1, 4))
        else:
            m["wqk"] = np.ascontiguousarray(
                wqk_t.astype(BF).reshape(8, 128, 8, 128).transpose(0, 2, 1, 3))
        in_maps.append(m)
    return in_maps


def kernel(x, W, Wo):
    from concourse.bass_utils import run_bass_kernel_spmd

    res = run_bass_kernel_spmd(_get_nc(), _in_maps(x, W, Wo),
                               list(range(NCORES)))
    out = np.empty((B, L, D), np.float32)
    for b in range(B):
        out[b] = (res.results[2 * b]["y"].astype(np.float32)
                  + res.results[2 * b + 1]["y"].astype(np.float32))
    return out


def _install_ntff_hook_shim():
    """The trimmed repo lacks antenv.axon_hooks; reconstruct it so
    run_bass_kernel_spmd(trace=True) can NTFF-profile through axon."""
    import sys as _sys, types
    if "antenv.axon_hooks" in _sys.modules:
        return
    import antenv  # noqa: F401
    from trn_agent_boot.trn_boot import _ntff_profile_via_ctypes
    hook = _ntff_profile_via_ctypes("/opt/axon/libaxon_pjrt.so")
    mod = types.ModuleType("antenv.axon_hooks")
    mod.set_axon_ntff_profile_hook = lambda h: None
    mod.get_axon_ntff_profile_hook = lambda: hook
    _sys.modules["antenv.axon_hooks"] = mod


def kernel_traced(x, W, Wo, tmpdir=None):
    """Run with NTFF tracing; returns BassKernelResults (trace in tmpdir)."""
    from concourse.bass_utils import run_bass_kernel_spmd

    _install_ntff_hook_shim()
    res = run_bass_kernel_spmd(_get_nc(), _in_maps(x, W, Wo),
                               list(range(NCORES)), trace=True, tmpdir=tmpdir)
    return res.exec_time_ns
